# revision 36
# baseline (speedup 1.0000x reference)
"""Trainium2 Bass kernel for nn_NeuromorphicNetwork (8-core SPMD).

Math: with REFRACT=1.0 and current_time = spike_count, after a neuron's first
spike (last=t0, count=t0+1) the gate `t - last > 1` evaluates to exactly 1 > 1
= False forever, so every neuron spikes AT MOST ONCE over the entire batch
scan, and threshold adaptation (count>10) never triggers.  Pre-first-spike the
membrane follows the unreset linear recurrence; over one batch item (10 steps,
constant current c): v' = lam^10 * v + g10 * c with g10 = (1-lam^10)/(1-lam),
and a spike occurs within the item iff v' >= 1 (the 10-step trajectory is
monotone between endpoints, entry v < 1).  So per neuron the whole scan
reduces to: w_b = alpha*w_{b-1} + d_b (d = raw current), first b with
w_b >= THR -> one-hot spike-rate row of value 0.1.

Sharding: tensor-parallel over hidden_dim.  Rate encoding sharded over
input_dim; fp8 spike counts are AllGathered in 2 chunks (128KB/rank each) so
GEMM1's first half overlaps the second AllGather.  Hidden spike one-hot pairs
(fT) stay in SBUF; GEMM2 contracts the LOCAL hidden slice against
W_ho[hid-slice, :] and a single fp8 ReduceScatter (payload scaled 1/32 to fit
e4m3) combines the output currents.  3 collectives total.

Per-core timeline (core m):
  stage A   : counts[i,b] = #{t: u[b,i,t] < sigmoid(x[b,i])} for its 512-wide
              input-dim slice; 10 per-t-plane bf16 is_lt on DVE + tree-sum
              split DVE/GPSIMD; fp8 counts -> AllGather half after tiles 1, 3.
  GEMM1     : fp8 DoubleRow matmuls, cur_hT[h,b] += W_ih-pair.T @ cnt-pair;
              half 0 r-outer/m-inner (overlaps AG half 1), half 1 m-outer so
              the 8 PSUM banks complete staggered ~2.4us apart.
  scan      : per hidden tile, chasing the half-1 stagger: w = scan(alpha,
              psum) [DVE], g = Sign(w-THR) [ACT], g*(iota-1024) [GPSIMD],
              min-reduce + first-crossing is_equal -> fp8 fT pair [DVE].
  GEMM2     : fp8 DoubleRow s-outer into 8 PSUM banks -- the first 3 s-terms
              interleave into tensor-engine gaps while scans finish; psum
              scaled 1/32 -> fp8 rs_in -> ReduceScatter(add).
  out layer : same scan + first-crossing on rs_out (THR/32), x0.1 -> res.
Host assembles out[b, o] from the 8 transposed slices.

HW notes (measured): the first collective cannot execute before ~70us after
kernel start (NRT model-switch floor; +~2.5us per extra collective in the
NEFF), so stage A hides under the floor and the design minimizes collective
count (3 total).  fp8 DoubleRow pairs run ~294-340ns (1.7x bf16).
"""

import sys
import numpy as np

for _p in ("/opt/trn_rl_repo", "/root/.axon_site/_ro/trn_rl_repo"):
    if _p not in sys.path:
        sys.path.insert(0, _p)

import concourse.bass as bass
import concourse.mybir as mybir
import concourse.tile as tile
from concourse.tile_rust import add_dep_helper
from concourse import bacc
from concourse.bass_utils import run_bass_kernel_spmd

F32 = mybir.dt.float32
BF16 = mybir.dt.bfloat16
F8 = mybir.dt.float8e4
AL = mybir.AluOpType
ACT = mybir.ActivationFunctionType
DR = mybir.MatmulPerfMode.DoubleRow

B = 512            # batch (free dim everywhere)
IN_DIM = 4096
HID = 8192
OUT = 1024
T = 10
NCORES = 8
IN_SL = IN_DIM // NCORES    # 512 input dims per core
H_SL = HID // NCORES        # 1024 hidden per core
O_SL = OUT // NCORES        # 128 outputs per core
P = 128
NCH = IN_SL // P            # 4 AllGather chunks of counts
MT = H_SL // P              # 8 hidden tiles per core

# exact scalar constants (float64 derivation, float32 use)
_LAM = np.float64(np.float32(0.95))
ALPHA = float(_LAM ** 10)                                # per-item decay
_G10 = float((1.0 - _LAM ** 10) / (1.0 - _LAM))          # per-item current gain
# true v = 0.1 * G10 * w  (w is the scan of raw count-currents); v >= 1 <=> w >= THR
THR = float(10.0 / _G10)
BIGB = 1024.0      # > any valid batch index sentinel offset


def _build_nc():
    nc = bacc.Bacc(num_devices=NCORES)

    xt = nc.declare_dram_parameter("xt", [IN_SL, B], BF16, isOutput=False)
    u = nc.declare_dram_parameter("u", [NCH, P, T, B], BF16, isOutput=False)
    # paired fp8 weights: [g*8+r, p, i, h] = W_ih[r*512 + g*256 + i*128 + p, hsl]
    w_ih = nc.declare_dram_parameter("w_ih", [16, P, 2, H_SL], F8, isOutput=False)
    # paired fp8 weights: [s, p, i, o] = W_ho[m*1024 + s*256 + i*128 + p, o]
    w_ho = nc.declare_dram_parameter("w_ho", [4, P, 2, OUT], F8, isOutput=False)
    res = nc.declare_dram_parameter("res", [O_SL, B], F32, isOutput=True)

    iota_np = np.broadcast_to(np.arange(B, dtype=np.float32), (P, B))
    iota_dram = nc.inline_tensor(np.ascontiguousarray(iota_np), name="iota_c")

    with tile.TileContext(nc, num_cores=NCORES) as tc:
        with (
            tc.tile_pool(name="const", bufs=1) as constp,
            tc.tile_pool(name="dram", bufs=1, space="DRAM") as dramp,
            tc.tile_pool(name="stgA", bufs=2) as apool,
            tc.tile_pool(name="ubuf", bufs=2) as upool,
            tc.tile_pool(name="wih", bufs=16) as wpool,
            tc.tile_pool(name="who", bufs=32) as wopool,
            tc.tile_pool(name="cpair", bufs=16) as cppool,
            tc.tile_pool(name="fpair", bufs=8) as fppool,
            tc.tile_pool(name="scan", bufs=2) as spool,
        ):
            # ---- constants ----
            iota_f = constp.tile([P, B], F32, name="iota_f")
            nc.sync.dma_start(iota_f, iota_dram[:, :])
            iotamB = constp.tile([P, B], F32, name="iotamB")
            nc.vector.tensor_scalar_add(iotamB, iota_f, -BIGB)
            alpha_1 = constp.tile([P, 1], F32, name="alpha_1")
            nc.vector.memset(alpha_1, ALPHA)
            alpha_t = alpha_1.to_broadcast((P, B))
            nthr = constp.tile([P, 1], F32, name="nthr")
            nc.vector.memset(nthr, -THR)
            iota_hi = constp.tile([P, B], F32, name="iota_hi")
            nc.vector.tensor_scalar_add(iota_hi, iota_f, BIGB)

            # W_ih half 0 streams from the start; W_ih half 1 + W_ho are
            # emitted behind the AllGather-gated pair loads in the scalar
            # queue so they don't compete with the encode stream.
            wih_sbs = []
            for k in range(16):
                w_sb = wpool.tile([P, 2, H_SL], F8, name=f"wih{k}", tag="wih")
                wih_sbs.append(w_sb)
                if k < 8:
                    nc.scalar.dma_start(w_sb, w_ih[k])
            who_sbs = [wopool.tile([P, 2, OUT], F8, name=f"who{j}", tag="who")
                       for j in range(4)]

            # ---- stage A: spike-count encoding, 2-chunk AllGather ----
            cnt_local = [dramp.tile([2 * P, B], F8, name=f"cntl{g}")
                         for g in range(2)]
            cnt_all = [dramp.tile([NCORES * 2 * P, B], F8, name=f"cnta{g}",
                                  addr_space="Shared") for g in range(2)]
            for c in range(NCH):
                xt_sb = apool.tile([P, B], BF16, name="xt_sb", tag="xt")
                nc.sync.dma_start(xt_sb, xt[c * P:(c + 1) * P, :])
                sig = apool.tile([P, B], BF16, name="sig", tag="sig")
                nc.scalar.activation(sig, xt_sb, ACT.Sigmoid)
                u_sb = upool.tile([P, T * B], BF16, name="u_sb", tag="u")
                nc.sync.dma_start(u_sb, u[c].rearrange("p t b -> p (t b)"))
                # per-t-plane compares against sig (2x bf16 DVE mode per op)
                cmp = upool.tile([P, T * B], BF16, name="cmp", tag="cmp")
                for t in range(T):
                    nc.vector.tensor_tensor(
                        cmp[:, t * B:(t + 1) * B], u_sb[:, t * B:(t + 1) * B],
                        sig, AL.is_lt)
                # tree-sum the 10 t-planes (integers <= 10, exact in bf16/fp8)
                s1 = apool.tile([P, 5 * B], BF16, name="s1", tag="s1")
                nc.vector.tensor_tensor(s1, cmp[:, :5 * B], cmp[:, 5 * B:], AL.add)
                s2 = apool.tile([P, 2 * B], BF16, name="s2", tag="s2")
                nc.gpsimd.tensor_tensor(s2, s1[:, :2 * B], s1[:, 2 * B:4 * B], AL.add)
                s3 = apool.tile([P, B], BF16, name="s3", tag="s3")
                nc.gpsimd.tensor_tensor(s3, s2[:, :B], s2[:, B:], AL.add)
                cnt8 = apool.tile([P, B], F8, name="cnt8", tag="cnt8")
                nc.gpsimd.tensor_tensor(cnt8, s3, s1[:, 4 * B:], AL.add)
                nc.gpsimd.dma_start(
                    cnt_local[c // 2][(c % 2) * P:(c % 2 + 1) * P, :], cnt8)
                if c % 2 == 1:
                    nc.gpsimd.collective_compute(
                        "AllGather", AL.bypass,
                        replica_groups=[list(range(NCORES))],
                        ins=[cnt_local[c // 2][:, :]],
                        outs=[cnt_all[c // 2][:, :]],
                    )

            with tc.tile_pool(name="psh", bufs=8, space="PSUM") as pshp:
                psum_h = [pshp.tile([P, B], F32, name=f"ph{m}", tag="ph")
                          for m in range(MT)]

                # pair (g, r): AG-half g's rows r*256..r*256+256 = rank r's two
                # 128-row count tiles -> DoubleRow rhs [p, i, b]
                def load_pair(g, r, eng=None):
                    pr = cppool.tile([P, 2, B], F8, name=f"cp{g}_{r}", tag="cp")
                    (eng or nc.scalar).dma_start(
                        pr, cnt_all[g][256 * r:256 * (r + 1), :]
                        .rearrange("(i p) b -> p i b", i=2))
                    return pr

                # ---- GEMM1 half 0: r-outer/m-inner once AG half 0 lands ----
                for r in range(NCORES):
                    pr = load_pair(0, r)
                    if r == 1:
                        # W_ih half 1 + W_ho: queued behind the AG0-gated pair
                        # load so they stream only after the encode phase
                        for k in range(8, 16):
                            nc.scalar.dma_start(wih_sbs[k], w_ih[k])
                        for j in range(4):
                            nc.scalar.dma_start(who_sbs[j], w_ho[j])
                    for m in range(MT):
                        nc.tensor.matmul(
                            psum_h[m],
                            lhsT=wih_sbs[r][:, :, m * P:(m + 1) * P],
                            rhs=pr,
                            start=(r == 0), stop=False,
                            perf_mode=DR,
                        )
                # ---- GEMM1 half 1: m-outer so banks complete staggered ----
                engs = [nc.scalar, nc.sync, nc.gpsimd]
                last_pairs = [load_pair(1, r, engs[r % 3])
                              for r in range(NCORES)]
                for m in range(MT):
                    for r in range(NCORES):
                        nc.tensor.matmul(
                            psum_h[m],
                            lhsT=wih_sbs[8 + r][:, :, m * P:(m + 1) * P],
                            rhs=last_pairs[r],
                            start=False, stop=(r == NCORES - 1),
                            perf_mode=DR,
                        )

                # ---- hidden layer: filter scan + first-crossing one-hot ----
                # one-hot trick: g = Sign(w-THR) in {-1,0,1}; m = min_b(g*(b-1024))
                # is (first_b - 1024) when a crossing exists (negative), else >= 0;
                # fT = (iota-1024 == m) is then the first-crossing one-hot.
                # fT pairs live in SBUF, laid out [128, 2, B] so GEMM2 can use
                # them directly as DoubleRow rhs operands.
                ft_pairs = [fppool.tile([P, 2, B], F8, name=f"ftp{j}", tag="fp")
                            for j in range(MT // 2)]
                for m in range(MT):
                    w_sc = spool.tile([P, B], F32, name="w_sc", tag="wsc")
                    nc.vector.tensor_tensor_scan(
                        w_sc, alpha_t, psum_h[m], 0.0, AL.mult, AL.add)
                    g = spool.tile([P, B], F32, name="g", tag="g")
                    nc.scalar.activation(g, w_sc, ACT.Sign, bias=nthr)
                    sc2 = spool.tile([P, B], F32, name="sc2", tag="sc2")
                    nc.gpsimd.tensor_tensor(sc2, g, iotamB, AL.mult)
                    negb = spool.tile([P, 1], F32, name="negb", tag="negb")
                    nc.vector.tensor_reduce(
                        negb, sc2, axis=mybir.AxisListType.X, op=AL.min)
                    nc.vector.tensor_scalar(
                        ft_pairs[m // 2][:, m % 2, :], iotamB, negb, None,
                        AL.is_equal)

            # ---- GEMM2: local hidden slice x W_ho, then one ReduceScatter ----
            # partial currents are scaled by 1/32 so the reduced SUM fits
            # fp8e4 (<= 8*1024/32 = 256 < 448); output layer uses THR/32.
            rs_in = dramp.tile([OUT, B], F8, name="rs_in")
            with tc.tile_pool(name="pso", bufs=8, space="PSUM") as psop:
                # s-outer: the first 3 s-terms only need early fT pairs, so
                # these matmuls slot into tensor-engine gaps while GEMM1
                # half 1 / the scans are still finishing; only the final 8
                # matmuls (s=3) wait for the last fT pair.
                psum_o = [psop.tile([P, B], F32, name=f"po{o}", tag="po")
                          for o in range(OUT // P)]
                for s in range(4):
                    for o in range(OUT // P):
                        nc.tensor.matmul(
                            psum_o[o],
                            lhsT=who_sbs[s][:, :, o * P:(o + 1) * P],
                            rhs=ft_pairs[s],
                            start=(s == 0), stop=(s == 3),
                            perf_mode=DR,
                        )
                        if s == 3:
                            ob = apool.tile([P, B], F8, name="ob", tag="ob",
                                            bufs=4)
                            nc.vector.tensor_scalar_mul(ob, psum_o[o], 0.03125)
                            if o % 2 == 0:
                                nc.gpsimd.dma_start(
                                    rs_in[o * P:(o + 1) * P, :], ob)
                            else:
                                nc.scalar.dma_start(
                                    rs_in[o * P:(o + 1) * P, :], ob)
                rs_out = dramp.tile([O_SL, B], F8, name="rs_out")
                nc.gpsimd.collective_compute(
                    "ReduceScatter", AL.add,
                    replica_groups=[list(range(NCORES))],
                    ins=[rs_in[:, :]], outs=[rs_out[:, :]],
                )

                # ---- output layer: same scan + first-crossing, scaled 0.1 ----
                ro = spool.tile([P, B], F8, name="ro", tag="ro")
                nc.sync.dma_start(ro, rs_out[:, :])
                wo = spool.tile([P, B], F32, name="wo", tag="wsc")
                nc.vector.tensor_tensor_scan(
                    wo, alpha_t, ro, 0.0, AL.mult, AL.add)
                g2 = spool.tile([P, B], F32, name="g2", tag="g")
                nc.vector.tensor_scalar(g2, wo, THR / 32.0, None, AL.is_ge)
                midx2 = spool.tile([P, B], F32, name="midx2", tag="sc2")
                nc.vector.scalar_tensor_tensor(
                    midx2, g2, -BIGB, iota_hi, AL.mult, AL.add)
                bmin2 = spool.tile([P, 1], F32, name="bmin2", tag="negb")
                nc.vector.tensor_reduce(
                    bmin2, midx2, axis=mybir.AxisListType.X, op=AL.min)
                out_sb = spool.tile([P, B], F32, name="out_sb", tag="outsb")
                nc.vector.tensor_scalar(
                    out_sb, iota_f, bmin2, float(np.float32(0.1)),
                    AL.is_equal, AL.mult)
                nc.sync.dma_start(res[:, :], out_sb)

    nc.finalize()
    return nc


_STATE = {}


def _get_uniforms():
    """The key-42 uniform draws the reference's bernoulli uses — input-independent
    constants. [B, IN_DIM, T] float32, computed once on host."""
    if "u" not in _STATE:
        import jax
        import jax.numpy as jnp
        f = jax.jit(lambda: jax.random.uniform(
            jax.random.key(42), (B, IN_DIM, T), jnp.float32), backend="cpu")
        _STATE["u"] = np.asarray(f())
    return _STATE["u"]


def _get_nc():
    if "nc" not in _STATE:
        _STATE["nc"] = _build_nc()
    return _STATE["nc"]


def make_in_maps(x, W_ih, W_ho):
    import ml_dtypes

    F8NP = ml_dtypes.float8_e4m3
    BF16NP = ml_dtypes.bfloat16
    x = np.ascontiguousarray(x, dtype=np.float32)
    W_ih = np.ascontiguousarray(W_ih, dtype=np.float32)
    W_ho = np.ascontiguousarray(W_ho, dtype=np.float32)
    u = _get_uniforms()

    in_maps = []
    for m in range(NCORES):
        isl = slice(m * IN_SL, (m + 1) * IN_SL)
        # u[b, i, t] -> [i_slice, t, b] -> [4, 128, T, B] bf16
        uc = np.ascontiguousarray(
            u[:, isl, :].transpose(1, 2, 0).reshape(NCH, P, T, B)
        ).astype(BF16NP)
        # W_ih[:, hsl] paired: [g, r, p, i, h], row = r*512 + g*256 + i*128 + p
        wih = (W_ih[:, m * H_SL:(m + 1) * H_SL]
               .reshape(8, 2, 2, P, H_SL)        # [r, g, i, p, h]
               .transpose(1, 0, 3, 2, 4)          # [g, r, p, i, h]
               .reshape(16, P, 2, H_SL))
        # W_ho[hid-slice, :] paired: [s, p, i, o] = W_ho[m*1024+s*256+i*128+p, o]
        who = (W_ho[m * H_SL:(m + 1) * H_SL, :]
               .reshape(4, 2, P, OUT)             # [s, i, p, o]
               .transpose(0, 2, 1, 3))            # [s, p, i, o]
        in_maps.append({
            "xt": np.ascontiguousarray(x[:, isl].T).astype(BF16NP),
            "u": uc,
            "w_ih": np.ascontiguousarray(wih).astype(F8NP),
            "w_ho": np.ascontiguousarray(who).astype(F8NP),
        })
    return in_maps


def assemble_out(results):
    out = np.empty((B, OUT), np.float32)
    for m in range(NCORES):
        out[:, m * O_SL:(m + 1) * O_SL] = results[m]["res"].T
    return out


def kernel(x, W_ih, W_ho):
    nc = _get_nc()
    in_maps = make_in_maps(x, W_ih, W_ho)
    r = run_bass_kernel_spmd(nc, in_maps, list(range(NCORES)))

    return assemble_out(r.results)


if __name__ == "__main__":
    # quick self-exercise with random inputs
    rng = np.random.default_rng(0)
    x = rng.standard_normal((B, IN_DIM), dtype=np.float32)
    W_ih = np.clip(0.5 + 0.1 * rng.standard_normal((IN_DIM, HID)), 0, 1).astype(np.float32)
    W_ho = np.clip(0.5 + 0.1 * rng.standard_normal((HID, OUT)), 0, 1).astype(np.float32)
    out = kernel(x, W_ih, W_ho)
    print("out", out.shape, out.dtype, "nonzero rows:", np.unique(np.nonzero(out)[0]))


# revision 37
# speedup vs baseline: 1.2464x; 1.2464x over previous
"""Trainium2 Bass kernel for nn_NeuromorphicNetwork (8-core SPMD).

Math: with REFRACT=1.0 and current_time = spike_count, after a neuron's first
spike (last=t0, count=t0+1) the gate `t - last > 1` evaluates to exactly 1 > 1
= False forever, so every neuron spikes AT MOST ONCE over the entire batch
scan, and threshold adaptation (count>10) never triggers.  Pre-first-spike the
membrane follows the unreset linear recurrence; over one batch item (10 steps,
constant current c): v' = lam^10 * v + g10 * c with g10 = (1-lam^10)/(1-lam),
and a spike occurs within the item iff v' >= 1 (the 10-step trajectory is
monotone between endpoints, entry v < 1).  So per neuron the whole scan
reduces to: w_b = alpha*w_{b-1} + d_b (d = raw current), first b with
w_b >= THR -> one-hot spike-rate row of value 0.1.

Sharding: tensor-parallel over hidden_dim.  Rate encoding sharded over
input_dim; fp8 spike counts are AllGathered in 2 chunks (128KB/rank each) so
GEMM1's first half overlaps the second AllGather.  Hidden spike one-hot pairs
(fT) stay in SBUF; GEMM2 contracts the LOCAL hidden slice against
W_ho[hid-slice, :] and a single fp8 ReduceScatter (payload scaled 1/32 to fit
e4m3) combines the output currents.  3 collectives total.

Per-core timeline (core m):
  stage A   : counts[i,b] = #{t: u[b,i,t] < sigmoid(x[b,i])} for its 512-wide
              input-dim slice; 10 per-t-plane bf16 is_lt on DVE + tree-sum
              split DVE/GPSIMD; fp8 counts -> AllGather half after tiles 1, 3.
  GEMM1     : fp8 DoubleRow matmuls, cur_hT[h,b] += W_ih-pair.T @ cnt-pair;
              half 0 r-outer/m-inner (overlaps AG half 1), half 1 m-outer so
              the 8 PSUM banks complete staggered ~2.4us apart.
  scan      : per hidden tile, chasing the half-1 stagger: w = scan(alpha,
              psum) [DVE], g = Sign(w-THR) [ACT], g*(iota-1024) [GPSIMD],
              min-reduce + first-crossing is_equal -> fp8 fT pair [DVE].
  GEMM2     : fp8 DoubleRow s-outer into 8 PSUM banks -- the first 3 s-terms
              interleave into tensor-engine gaps while scans finish; psum
              scaled 1/32 -> fp8 rs_in -> ReduceScatter(add).
  out layer : same scan + first-crossing on rs_out (THR/32), x0.1 -> res.
Host assembles out[b, o] from the 8 transposed slices.

HW notes (measured): the first collective cannot execute before ~70us after
kernel start (NRT model-switch floor; +~2.5us per extra collective in the
NEFF), so stage A hides under the floor and the design minimizes collective
count (3 total).  fp8 DoubleRow pairs run ~294-340ns (1.7x bf16).
"""

import sys
import numpy as np

for _p in ("/opt/trn_rl_repo", "/root/.axon_site/_ro/trn_rl_repo"):
    if _p not in sys.path:
        sys.path.insert(0, _p)

import concourse.bass as bass
import concourse.mybir as mybir
import concourse.tile as tile
from concourse.tile_rust import add_dep_helper
from concourse import bacc
from concourse.bass_utils import run_bass_kernel_spmd

F32 = mybir.dt.float32
BF16 = mybir.dt.bfloat16
F8 = mybir.dt.float8e4
AL = mybir.AluOpType
ACT = mybir.ActivationFunctionType
DR = mybir.MatmulPerfMode.DoubleRow

B = 512            # batch (free dim everywhere)
IN_DIM = 4096
HID = 8192
OUT = 1024
T = 10
NCORES = 8
IN_SL = IN_DIM // NCORES    # 512 input dims per core
H_SL = HID // NCORES        # 1024 hidden per core
O_SL = OUT // NCORES        # 128 outputs per core
P = 128
NCH = IN_SL // P            # 4 AllGather chunks of counts
MT = H_SL // P              # 8 hidden tiles per core

# exact scalar constants (float64 derivation, float32 use)
_LAM = np.float64(np.float32(0.95))
ALPHA = float(_LAM ** 10)                                # per-item decay
_G10 = float((1.0 - _LAM ** 10) / (1.0 - _LAM))          # per-item current gain
# true v = 0.1 * G10 * w  (w is the scan of raw count-currents); v >= 1 <=> w >= THR
THR = float(10.0 / _G10)
BIGB = 1024.0      # > any valid batch index sentinel offset


def _build_nc():
    nc = bacc.Bacc(num_devices=NCORES)

    xt = nc.declare_dram_parameter("xt", [IN_SL, B], BF16, isOutput=False)
    u = nc.declare_dram_parameter("u", [NCH, P, T, B], BF16, isOutput=False)
    # paired fp8 weights: [g*8+r, p, i, h] = W_ih[r*512 + g*256 + i*128 + p, hsl]
    w_ih = nc.declare_dram_parameter("w_ih", [16, P, 2, H_SL], F8, isOutput=False)
    # paired fp8 weights: [s, p, i, o] = W_ho[m*1024 + s*256 + i*128 + p, o]
    w_ho = nc.declare_dram_parameter("w_ho", [4, P, 2, OUT], F8, isOutput=False)
    res = nc.declare_dram_parameter("res", [O_SL, B], F32, isOutput=True)

    iota_np = np.broadcast_to(np.arange(B, dtype=np.float32), (P, B))
    iota_dram = nc.inline_tensor(np.ascontiguousarray(iota_np), name="iota_c")

    with tile.TileContext(nc, num_cores=NCORES) as tc:
        with (
            tc.tile_pool(name="const", bufs=1) as constp,
            tc.tile_pool(name="dram", bufs=1, space="DRAM") as dramp,
            tc.tile_pool(name="stgA", bufs=2) as apool,
            tc.tile_pool(name="ubuf", bufs=2) as upool,
            tc.tile_pool(name="wih", bufs=16) as wpool,
            tc.tile_pool(name="who", bufs=32) as wopool,
            tc.tile_pool(name="cpair", bufs=8) as cppool,
            tc.tile_pool(name="fpair", bufs=8) as fppool,
            tc.tile_pool(name="scan", bufs=2) as spool,
        ):
            # ---- constants ----
            iota_f = constp.tile([P, B], F32, name="iota_f")
            nc.sync.dma_start(iota_f, iota_dram[:, :])
            iotamB = constp.tile([P, B], F32, name="iotamB")
            nc.vector.tensor_scalar_add(iotamB, iota_f, -BIGB)
            alpha_t = constp.tile([P, B], F32, name="alpha_t")
            nc.vector.memset(alpha_t, ALPHA)
            nthr = constp.tile([P, 1], F32, name="nthr")
            nc.vector.memset(nthr, -THR)
            iota_hi = constp.tile([P, B], F32, name="iota_hi")
            nc.vector.tensor_scalar_add(iota_hi, iota_f, BIGB)

            # W_ih half 0 streams from the start; W_ih half 1 + W_ho are
            # emitted behind the AllGather-gated pair loads in the scalar
            # queue so they don't compete with the encode stream.
            wih_sbs = []
            for k in range(16):
                w_sb = wpool.tile([P, 2, H_SL], F8, name=f"wih{k}", tag="wih")
                wih_sbs.append(w_sb)
                if k < 8:
                    nc.scalar.dma_start(w_sb, w_ih[k])
            who_sbs = [wopool.tile([P, 2, OUT], F8, name=f"who{j}", tag="who")
                       for j in range(4)]

            # ---- stage A: spike-count encoding, 2-chunk AllGather ----
            cnt_local = [dramp.tile([2 * P, B], F8, name=f"cntl{g}")
                         for g in range(2)]
            cnt_all = [dramp.tile([NCORES * 2 * P, B], F8, name=f"cnta{g}",
                                  addr_space="Shared") for g in range(2)]
            for c in range(NCH):
                xt_sb = apool.tile([P, B], BF16, name="xt_sb", tag="xt")
                nc.sync.dma_start(xt_sb, xt[c * P:(c + 1) * P, :])
                sig = apool.tile([P, B], BF16, name="sig", tag="sig")
                nc.scalar.activation(sig, xt_sb, ACT.Sigmoid)
                u_sb = upool.tile([P, T * B], BF16, name="u_sb", tag="u")
                nc.sync.dma_start(u_sb, u[c].rearrange("p t b -> p (t b)"))
                # per-t-plane compares against sig (2x bf16 DVE mode per op)
                cmp = upool.tile([P, T * B], BF16, name="cmp", tag="cmp")
                for t in range(T):
                    nc.vector.tensor_tensor(
                        cmp[:, t * B:(t + 1) * B], u_sb[:, t * B:(t + 1) * B],
                        sig, AL.is_lt)
                # tree-sum the 10 t-planes (integers <= 10, exact in bf16/fp8)
                s1 = apool.tile([P, 5 * B], BF16, name="s1", tag="s1")
                nc.vector.tensor_tensor(s1, cmp[:, :5 * B], cmp[:, 5 * B:], AL.add)
                s2 = apool.tile([P, 2 * B], BF16, name="s2", tag="s2")
                nc.gpsimd.tensor_tensor(s2, s1[:, :2 * B], s1[:, 2 * B:4 * B], AL.add)
                s3 = apool.tile([P, B], BF16, name="s3", tag="s3")
                nc.gpsimd.tensor_tensor(s3, s2[:, :B], s2[:, B:], AL.add)
                cnt8 = apool.tile([P, B], F8, name="cnt8", tag="cnt8")
                nc.gpsimd.tensor_tensor(cnt8, s3, s1[:, 4 * B:], AL.add)
                nc.gpsimd.dma_start(
                    cnt_local[c // 2][(c % 2) * P:(c % 2 + 1) * P, :], cnt8)
                if c % 2 == 1:
                    nc.gpsimd.collective_compute(
                        "AllGather", AL.bypass,
                        replica_groups=[list(range(NCORES))],
                        ins=[cnt_local[c // 2][:, :]],
                        outs=[cnt_all[c // 2][:, :]],
                    )

            with tc.tile_pool(name="psh", bufs=8, space="PSUM") as pshp:
                psum_h = [pshp.tile([P, B], F32, name=f"ph{m}", tag="ph")
                          for m in range(MT)]

                # pair (g, r): AG-half g's rows r*256..r*256+256 = rank r's two
                # 128-row count tiles -> DoubleRow rhs [p, i, b]
                def load_pair(g, r):
                    pr = cppool.tile([P, 2, B], F8, name=f"cp{g}_{r}", tag="cp")
                    nc.scalar.dma_start(
                        pr, cnt_all[g][256 * r:256 * (r + 1), :]
                        .rearrange("(i p) b -> p i b", i=2))
                    return pr

                # ---- GEMM1 half 0: r-outer/m-inner once AG half 0 lands ----
                for r in range(NCORES):
                    pr = load_pair(0, r)
                    if r == 1:
                        # W_ih half 1 + W_ho: queued behind the AG0-gated pair
                        # load so they stream only after the encode phase
                        for k in range(8, 16):
                            nc.scalar.dma_start(wih_sbs[k], w_ih[k])
                        for j in range(4):
                            nc.scalar.dma_start(who_sbs[j], w_ho[j])
                    for m in range(MT):
                        nc.tensor.matmul(
                            psum_h[m],
                            lhsT=wih_sbs[r][:, :, m * P:(m + 1) * P],
                            rhs=pr,
                            start=(r == 0), stop=False,
                            perf_mode=DR,
                        )
                # ---- GEMM1 half 1: m-outer so banks complete staggered ----
                last_pairs = [load_pair(1, r) for r in range(NCORES)]
                for m in range(MT):
                    for r in range(NCORES):
                        nc.tensor.matmul(
                            psum_h[m],
                            lhsT=wih_sbs[8 + r][:, :, m * P:(m + 1) * P],
                            rhs=last_pairs[r],
                            start=False, stop=(r == NCORES - 1),
                            perf_mode=DR,
                        )

                # ---- hidden layer: filter scan + first-crossing one-hot ----
                # one-hot trick: g = Sign(w-THR) in {-1,0,1}; m = min_b(g*(b-1024))
                # is (first_b - 1024) when a crossing exists (negative), else >= 0;
                # fT = (iota-1024 == m) is then the first-crossing one-hot.
                # fT pairs live in SBUF, laid out [128, 2, B] so GEMM2 can use
                # them directly as DoubleRow rhs operands.
                ft_pairs = [fppool.tile([P, 2, B], F8, name=f"ftp{j}", tag="fp")
                            for j in range(MT // 2)]
                for m in range(MT):
                    w_sc = spool.tile([P, B], F32, name="w_sc", tag="wsc")
                    nc.vector.tensor_tensor_scan(
                        w_sc, alpha_t, psum_h[m], 0.0, AL.mult, AL.add)
                    g = spool.tile([P, B], F32, name="g", tag="g")
                    nc.scalar.activation(g, w_sc, ACT.Sign, bias=nthr)
                    sc2 = spool.tile([P, B], F32, name="sc2", tag="sc2")
                    nc.gpsimd.tensor_tensor(sc2, g, iotamB, AL.mult)
                    negb = spool.tile([P, 1], F32, name="negb", tag="negb")
                    nc.vector.tensor_reduce(
                        negb, sc2, axis=mybir.AxisListType.X, op=AL.min)
                    nc.vector.tensor_scalar(
                        ft_pairs[m // 2][:, m % 2, :], iotamB, negb, None,
                        AL.is_equal)

            # ---- GEMM2: local hidden slice x W_ho, then one ReduceScatter ----
            # partial currents are scaled by 1/32 so the reduced SUM fits
            # fp8e4 (<= 8*1024/32 = 256 < 448); output layer uses THR/32.
            rs_in = dramp.tile([OUT, B], F8, name="rs_in")
            with tc.tile_pool(name="pso", bufs=8, space="PSUM") as psop:
                # s-outer: the first 3 s-terms only need early fT pairs, so
                # these matmuls slot into tensor-engine gaps while GEMM1
                # half 1 / the scans are still finishing; only the final 8
                # matmuls (s=3) wait for the last fT pair.
                psum_o = [psop.tile([P, B], F32, name=f"po{o}", tag="po")
                          for o in range(OUT // P)]
                for s in range(4):
                    for o in range(OUT // P):
                        nc.tensor.matmul(
                            psum_o[o],
                            lhsT=who_sbs[s][:, :, o * P:(o + 1) * P],
                            rhs=ft_pairs[s],
                            start=(s == 0), stop=(s == 3),
                            perf_mode=DR,
                        )
                        if s == 3:
                            ob = apool.tile([P, B], F8, name="ob", tag="ob",
                                            bufs=4)
                            nc.vector.tensor_scalar_mul(ob, psum_o[o], 0.03125)
                            if o % 2 == 0:
                                nc.gpsimd.dma_start(
                                    rs_in[o * P:(o + 1) * P, :], ob)
                            else:
                                nc.scalar.dma_start(
                                    rs_in[o * P:(o + 1) * P, :], ob)
                rs_out = dramp.tile([O_SL, B], F8, name="rs_out")
                nc.gpsimd.collective_compute(
                    "ReduceScatter", AL.add,
                    replica_groups=[list(range(NCORES))],
                    ins=[rs_in[:, :]], outs=[rs_out[:, :]],
                )

                # ---- output layer: same scan + first-crossing, scaled 0.1 ----
                ro = spool.tile([P, B], F8, name="ro", tag="ro")
                nc.sync.dma_start(ro, rs_out[:, :])
                wo = spool.tile([P, B], F32, name="wo", tag="wsc")
                nc.vector.tensor_tensor_scan(
                    wo, alpha_t, ro, 0.0, AL.mult, AL.add)
                g2 = spool.tile([P, B], F32, name="g2", tag="g")
                nc.vector.tensor_scalar(g2, wo, THR / 32.0, None, AL.is_ge)
                midx2 = spool.tile([P, B], F32, name="midx2", tag="sc2")
                nc.vector.scalar_tensor_tensor(
                    midx2, g2, -BIGB, iota_hi, AL.mult, AL.add)
                bmin2 = spool.tile([P, 1], F32, name="bmin2", tag="negb")
                nc.vector.tensor_reduce(
                    bmin2, midx2, axis=mybir.AxisListType.X, op=AL.min)
                out_sb = spool.tile([P, B], F32, name="out_sb", tag="outsb")
                nc.vector.tensor_scalar(
                    out_sb, iota_f, bmin2, float(np.float32(0.1)),
                    AL.is_equal, AL.mult)
                nc.sync.dma_start(res[:, :], out_sb)

    nc.finalize()
    return nc


_STATE = {}


def _get_uniforms():
    """The key-42 uniform draws the reference's bernoulli uses — input-independent
    constants. [B, IN_DIM, T] float32, computed once on host."""
    if "u" not in _STATE:
        import jax
        import jax.numpy as jnp
        f = jax.jit(lambda: jax.random.uniform(
            jax.random.key(42), (B, IN_DIM, T), jnp.float32), backend="cpu")
        _STATE["u"] = np.asarray(f())
    return _STATE["u"]


def _get_nc():
    if "nc" not in _STATE:
        _STATE["nc"] = _build_nc()
    return _STATE["nc"]


def make_in_maps(x, W_ih, W_ho):
    import ml_dtypes

    F8NP = ml_dtypes.float8_e4m3
    BF16NP = ml_dtypes.bfloat16
    x = np.ascontiguousarray(x, dtype=np.float32)
    W_ih = np.ascontiguousarray(W_ih, dtype=np.float32)
    W_ho = np.ascontiguousarray(W_ho, dtype=np.float32)
    u = _get_uniforms()

    in_maps = []
    for m in range(NCORES):
        isl = slice(m * IN_SL, (m + 1) * IN_SL)
        # u[b, i, t] -> [i_slice, t, b] -> [4, 128, T, B] bf16
        uc = np.ascontiguousarray(
            u[:, isl, :].transpose(1, 2, 0).reshape(NCH, P, T, B)
        ).astype(BF16NP)
        # W_ih[:, hsl] paired: [g, r, p, i, h], row = r*512 + g*256 + i*128 + p
        wih = (W_ih[:, m * H_SL:(m + 1) * H_SL]
               .reshape(8, 2, 2, P, H_SL)        # [r, g, i, p, h]
               .transpose(1, 0, 3, 2, 4)          # [g, r, p, i, h]
               .reshape(16, P, 2, H_SL))
        # W_ho[hid-slice, :] paired: [s, p, i, o] = W_ho[m*1024+s*256+i*128+p, o]
        who = (W_ho[m * H_SL:(m + 1) * H_SL, :]
               .reshape(4, 2, P, OUT)             # [s, i, p, o]
               .transpose(0, 2, 1, 3))            # [s, p, i, o]
        in_maps.append({
            "xt": np.ascontiguousarray(x[:, isl].T).astype(BF16NP),
            "u": uc,
            "w_ih": np.ascontiguousarray(wih).astype(F8NP),
            "w_ho": np.ascontiguousarray(who).astype(F8NP),
        })
    return in_maps


def assemble_out(results):
    out = np.empty((B, OUT), np.float32)
    for m in range(NCORES):
        out[:, m * O_SL:(m + 1) * O_SL] = results[m]["res"].T
    return out


def kernel(x, W_ih, W_ho):
    nc = _get_nc()
    in_maps = make_in_maps(x, W_ih, W_ho)
    r = run_bass_kernel_spmd(nc, in_maps, list(range(NCORES)))

    return assemble_out(r.results)


if __name__ == "__main__":
    # quick self-exercise with random inputs
    rng = np.random.default_rng(0)
    x = rng.standard_normal((B, IN_DIM), dtype=np.float32)
    W_ih = np.clip(0.5 + 0.1 * rng.standard_normal((IN_DIM, HID)), 0, 1).astype(np.float32)
    W_ho = np.clip(0.5 + 0.1 * rng.standard_normal((HID, OUT)), 0, 1).astype(np.float32)
    out = kernel(x, W_ih, W_ho)
    print("out", out.shape, out.dtype, "nonzero rows:", np.unique(np.nonzero(out)[0]))
